# revision 11
# baseline (speedup 1.0000x reference)
"""Multi-head self-attention (B=2, S=2048, D=1024, H=16, causal) on 8 TRN2 cores.

Sharding: tensor-parallel over heads. Core c owns heads {2c, 2c+1}:
  - Wq/Wk/Wv column-sharded: core c gets columns [128c, 128c+128).
  - All matmul inputs are bf16 (f32 PSUM accumulation): rel-err ~4e-3 vs the
    f32 reference, well inside the 2e-2 gate, and it halves DMA/SBUF traffic.
  - Each core computes Q^T,K^T,V^T (head-dim on partitions) for its heads,
    both batches; V is PE-transposed back to seq-on-partitions layout.
  - Attention in transposed-scores layout: S^T[k, q] tiles, so softmax
    denominators come free from an extra ones-column in V (row 64 of the
    AV psum accumulates sum_k P^T[k, q]).  Causal masking is a 0/1 vector
    multiply on the diagonal P^T tiles (DVE) instead of a -BIG mask matmul.
  - Z^T is exchanged via TWO AllToAlls so the output projection overlaps
    with attention compute: batch-0 Z (8 pieces of [128 x 256]) right after
    batch-0 attention — its output projection weaves into batch-1 attention;
    batch-1 Z at the very end, its collective latency covered by the
    remaining batch-0 projection chains.  Core p receives tokens
    [512(p//2)+256(p%2), +256) of each batch; it projects them with full Wo
    rows (no all-reduce); out rows = [256 b0 | 256 b1].
"""

import ml_dtypes
import numpy as np

import concourse.bass as bass
import concourse.mybir as mybir
import concourse.tile as tile
from concourse import bacc
from concourse.bass_utils import run_bass_kernel_spmd

N_CORES = 8
B, S, D = 2, 2048, 1024
H = 16
HD = D // H          # 64
BS = B * S           # 4096 flattened tokens
CD = 2 * HD          # 128 head-dims per core
NM = S // 512        # 4 q-chunks per batch
SCALE = 1.0 / np.sqrt(HD)

F32 = mybir.dt.float32
BF16 = mybir.dt.bfloat16
F32R = mybir.dt.float32r
EXP = mybir.ActivationFunctionType.Exp

_CACHE = {}


def build_nc(with_collective=True, reps=1):
    nc = bacc.Bacc("TRN2", target_bir_lowering=False, debug=False, num_devices=N_CORES)

    xT = nc.dram_tensor("xT", [D, BS], BF16, kind="ExternalInput").ap()
    wq = nc.dram_tensor("wq", [D, CD], BF16, kind="ExternalInput").ap()
    wk = nc.dram_tensor("wk", [D, CD], BF16, kind="ExternalInput").ap()
    wv = nc.dram_tensor("wv", [D, CD], BF16, kind="ExternalInput").ap()
    wo = nc.dram_tensor("wo", [D, D], BF16, kind="ExternalInput").ap()
    bo = nc.dram_tensor("bo", [1, D], F32, kind="ExternalInput").ap()
    ident = nc.dram_tensor("ident", [128, 128], BF16, kind="ExternalInput").ap()
    triu01 = nc.dram_tensor("triu01", [128, 128], BF16, kind="ExternalInput").ap()
    onesr = nc.dram_tensor("onesr", [128, 128], F32, kind="ExternalInput").ap()
    out = nc.dram_tensor("out", [512, D], F32, kind="ExternalOutput").ap()

    with tile.TileContext(nc) as tc:
        with (
            tc.tile_pool(name="const", bufs=1) as constp,
            tc.tile_pool(name="persist", bufs=1) as persist,
            tc.tile_pool(name="xt", bufs=2) as xtp,
            tc.tile_pool(name="work", bufs=3) as work,
            tc.tile_pool(name="dram", bufs=1, space="DRAM") as dram,
        ):
            # ---- small constants (loaded once) ----
            ident_sb = constp.tile([128, 128], BF16)
            triu_sb = constp.tile([128, 128], BF16)
            ones_sb = constp.tile([128, 128], F32R)
            onesb_sb = constp.tile([128, 64], BF16)
            cc_in0 = dram.tile([8, 128, 256], BF16)
            cc_out0 = dram.tile([8, 128, 256], BF16)
            cc_in1 = dram.tile([8, 128, 256], BF16)
            cc_out1 = dram.tile([8, 128, 256], BF16)
            xTr = xT.rearrange("(e p) s -> p e s", p=128)

            for _rep in range(reps):
                _body(nc, tc, constp, persist, xtp, work, dram,
                      xTr, wq, wk, wv, wo, bo, out,
                      ident_sb, triu_sb, ones_sb, onesb_sb,
                      (cc_in0, cc_out0, cc_in1, cc_out1),
                      with_collective, (ident, triu01, onesr),
                      first=(_rep == 0))

    nc.compile()
    return nc


def _body(nc, tc, constp, persist, xtp, work, dram,
          xTr, wq, wk, wv, wo, bo, out,
          ident_sb, triu_sb, ones_sb, onesb_sb, ccs,
          with_collective, const_srcs, first=True):
    cc_in0, cc_out0, cc_in1, cc_out1 = ccs
    r = F32R

    # ---- projection weights ----
    wq_sb = constp.tile([128, 8, CD], BF16, tag="wq", name="wq_sb")
    wk_sb = constp.tile([128, 8, CD], BF16, tag="wk", name="wk_sb")
    wv_sb = constp.tile([128, 8, CD], BF16, tag="wv", name="wv_sb")
    wqr = wq.rearrange("(e p) c -> p e c", p=128)
    # stage only the first contraction block of Wq so the first matmul can
    # start as soon as ~64KB has landed; the rest follows right behind
    nc.sync.dma_start(wq_sb[:, 0:1, :], wqr[:, 0:1, :])

    # ---- persistent activations ----
    qt_sb = persist.tile([128, BS], BF16, tag="qt", name="qt_sb")
    kt_sb = persist.tile([128, BS], BF16, tag="kt", name="kt_sb")
    v_sb = persist.tile([128, 32, 130], BF16, tag="v", name="v_sb")

    P = {}  # current-phase psum pools

    def v_transposes(sc, vt_t):
        for st in range(4):
            tt = 4 * sc + st
            v_ps = P["pp"].tile([128, 128], BF16, tag=P["pptag"], name=f"vtp{sc}{st}")
            nc.tensor.transpose(
                v_ps[:], vt_t[:, 128 * st:128 * st + 128], ident_sb[:],
            )
            nc.vector.tensor_copy(v_sb[:, tt, 0:64], v_ps[:, 0:64])
            nc.vector.tensor_copy(v_sb[:, tt, 65:129], v_ps[:, 64:128])

    def proj_parts(sc):
        """Yield fine-grained projection closures for one 512-token chunk."""
        sl = bass.ts(sc, 512)
        state = {}

        def load():
            xt_t = xtp.tile([128, 8, 512], BF16, tag="xt", name=f"xt{sc}")
            if sc == 0:
                # interleave x and weight slices so the Q chain streams
                wkr = wk.rearrange("(e p) c -> p e c", p=128)
                wvr = wv.rearrange("(e p) c -> p e c", p=128)
                nc.sync.dma_start(xt_t[:, 0:1, :], xTr[:, 0:1, sl])
                nc.sync.dma_start(wq_sb[:, 1:8, :], wqr[:, 1:8, :])
                nc.sync.dma_start(xt_t[:, 1:4, :], xTr[:, 1:4, sl])
                nc.sync.dma_start(wk_sb[:, 0:1, :], wkr[:, 0:1, :])
                nc.sync.dma_start(wv_sb[:, 0:1, :], wvr[:, 0:1, :])
                nc.sync.dma_start(xt_t[:, 4:8, :], xTr[:, 4:8, sl])
                nc.sync.dma_start(wk_sb[:, 1:8, :], wkr[:, 1:8, :])
                nc.sync.dma_start(wv_sb[:, 1:8, :], wvr[:, 1:8, :])
                if first:
                    ident_d, triu_d, onesr_d = const_srcs
                    nc.sync.dma_start(ident_sb[:], ident_d)
                    nc.sync.dma_start(triu_sb[:], triu_d)
                    nc.sync.dma_start(ones_sb[:], onesr_d.bitcast(r))
                    nc.vector.tensor_copy(onesb_sb[:], ones_sb[:, 0:64])
                    nc.vector.tensor_copy(v_sb[:, :, 64], onesb_sb[:, 0:32])
                    nc.vector.tensor_copy(v_sb[:, :, 129], onesb_sb[:, 0:32])
            else:
                nc.sync.dma_start(xt_t[:, 0:4, :], xTr[:, 0:4, sl])
                nc.sync.dma_start(xt_t[:, 4:8, :], xTr[:, 4:8, sl])
            state["xt"] = xt_t
            state["vt"] = xtp.tile([128, 512], BF16, tag="vtc", name=f"vtc{sc}")

        def group(w_sb, o_ap_fn, name):
            def run():
                p_ps = P["pp"].tile([128, 512], F32, tag=P["pptag"], name=f"pp{sc}{name}")
                for e in range(8):
                    nc.tensor.matmul(
                        p_ps[:], w_sb[:, e, :], state["xt"][:, e, :],
                        start=(e == 0), stop=(e == 7),
                    )
                with nc.allow_low_precision(reason="bf16 activations within tolerance"):
                    nc.vector.tensor_copy(o_ap_fn(), p_ps[:])
            return run

        yield load
        yield group(wq_sb, lambda: qt_sb[:, sl], "q")
        yield group(wk_sb, lambda: kt_sb[:, sl], "k")
        yield group(wv_sb, lambda: state["vt"][:], "v")
        yield lambda: v_transposes(sc, state["vt"][:])

    def proj_chunk(sc):
        for part in proj_parts(sc):
            part()

    def attn_chunk_beats(b, m, stream):
        """Yield one closure per beat; caller weaves streams together."""
        q0 = 2048 * b + 512 * m
        last_t = 4 * m + 3
        state = {}

        def beat(t):
            if t == 0:
                state["z"] = [
                    P["pz"].tile([65, 512], F32, tag=f"z{stream}{h}",
                                 name=f"z{b}{m}{h}", bufs=1)
                    for h in (0, 1)
                ]
            z_ps = state["z"]

            def av(ta, pt_sb):
                joa = max(0, 128 * (ta - 4 * m))
                for h in (0, 1):
                    nc.tensor.matmul(
                        z_ps[h][:, joa:512],
                        v_sb[:, 16 * b + ta, 65 * h:65 * h + 65],
                        pt_sb[:, 512 * h + joa:512 * h + 512],
                        start=(ta == 0), stop=(ta == last_t),
                    )

            k0 = 2048 * b + 128 * t
            jo = max(0, 128 * (t - 4 * m))
            pt_sb = work.tile([128, 1024], BF16, tag="pt", name=f"pt{b}{m}{t}", bufs=6)
            s_tiles = [
                P["ps"].tile([128, 512], F32, tag=f"s{h}", name=f"s{b}{m}{t}{h}")
                for h in (0, 1)
            ]
            for h in (0, 1):
                hsl = slice(64 * h, 64 * h + 64)
                nc.tensor.matmul(
                    s_tiles[h][:, jo:512],
                    kt_sb[hsl, k0:k0 + 128],
                    qt_sb[hsl, q0 + jo:q0 + 512],
                    start=True, stop=True,
                )
                with nc.allow_low_precision(reason="bf16 attn probs within tolerance"):
                    nc.scalar.activation(
                        pt_sb[:, 512 * h + jo:512 * h + 512], s_tiles[h][:, jo:512],
                        EXP, scale=float(SCALE),
                    )
                if t >= 4 * m:
                    # causal mask on the diagonal 128x128 tile: zero k>q
                    with nc.allow_low_precision(reason="bf16 attn probs"):
                        nc.vector.tensor_mul(
                            pt_sb[:, 512 * h + jo:512 * h + jo + 128],
                            pt_sb[:, 512 * h + jo:512 * h + jo + 128],
                            triu_sb[:],
                        )
            pend = state.setdefault("pend", [])
            pend.append((t, pt_sb))
            if len(pend) > 2:
                av(*pend.pop(0))
            if t == last_t:
                while pend:
                    av(*pend.pop(0))
                # late batch-1 norms use the PE-broadcast path: a DMA
                # round-trip there would delay the tail collective
                _norm(b, m, z_ps, fast=(b == 1 and m >= 2))

        for t in range(last_t + 1):
            yield lambda t=t: beat(t)

    def _norm(b, m, z_ps, fast=False):
        # normalize and stage for all-to-all; copy psum out (incl. denom row)
        # immediately to release the z banks, then finish from SBUF
        zcp = [work.tile([65, 512], F32, tag=f"zc{h}", name=f"zc{b}{m}{h}", bufs=2)
               for h in (0, 1)]
        for h in (0, 1):
            nc.vector.tensor_copy(zcp[h][:], z_ps[h][:])
        zt_sb = work.tile([128, 512], BF16, tag="zt", name=f"zt{b}{m}", bufs=2)
        for h in (0, 1):
            recip = work.tile([65, 512], r, tag="rc", name=f"rc{b}{m}{h}", bufs=2)
            with nc.allow_low_precision(reason="f32r is bitwise f32 here"):
                nc.vector.reciprocal(recip[64:65, :], zcp[h][64:65, :].bitcast(r))
            bc_sb = work.tile([64, 512], F32, tag="bc", name=f"bcs{b}{m}{h}", bufs=2)
            if fast:
                bc_ps = P["bc"].tile([64, 512], F32, tag=P["bctag"], name=f"bcp{b}{m}{h}")
                nc.tensor.matmul(
                    bc_ps[:], ones_sb[64:65, 0:64], recip[64:65, :],
                    start=True, stop=True,
                )
                nc.vector.tensor_copy(bc_sb[:], bc_ps[:])
            else:
                r_dram = dram.tile([1, 512], F32, tag="rd", name=f"rd{b}{m}{h}", bufs=2)
                nc.sync.dma_start(r_dram[:], recip[64:65, :].bitcast(F32))
                nc.sync.dma_start(bc_sb[:], r_dram.broadcast_to([64, 512]))
            with nc.allow_low_precision(reason="bf16 z within tolerance"):
                nc.vector.tensor_mul(
                    zt_sb[64 * h:64 * h + 64, :], zcp[h][0:64, :], bc_sb[:]
                )
        # stage pieces for the all-to-all
        cc = cc_in0 if b == 0 else cc_in1
        nc.sync.dma_start(
            cc[2 * m:2 * m + 2].rearrange("u p s -> p u s"),
            zt_sb[:].rearrange("p (u s) -> p u s", u=2),
        )

    def a2a(cin, cout):
        if with_collective:
            nc.gpsimd.collective_compute(
                "AllToAll",
                mybir.AluOpType.bypass,
                replica_groups=[list(range(N_CORES))],
                ins=[cin.opt()],
                outs=[cout.opt()],
            )
        else:
            nc.sync.dma_start(cout[:], cin[:])

    # ---- output projection pieces ----
    wo_sb = persist.tile([128, 8, D], BF16, tag="wo", name="wo_sb")
    bo_bc = constp.tile([128, D], F32, tag="bobc", name="bo_bc")

    def oproj_chain(zt2, st, e, psname):
        o_ps = P["po"].tile([128, 512], F32, tag="o", name=psname)
        for i in range(8):
            nc.tensor.matmul(
                o_ps[:],
                zt2[:, i, 128 * st:128 * st + 128],
                wo_sb[:, i, bass.ts(e, 512)],
                start=(i == 0), stop=(i == 7),
            )
        return o_ps

    def run_weave(beats, fillers):
        fi = iter(fillers)
        for bt in beats:
            bt()
            f = next(fi, None)
            if f is not None:
                f()
        for f in fi:
            if f is not None:
                f()

    def proj_fillers_a():
        for sc in range(1, 8):
            yield from proj_parts(sc)
        yield lambda: nc.sync.dma_start(
            wo_sb[:], wo.rearrange("(i p) e -> p i e", p=128))
        yield lambda: (nc.sync.dma_start(bo_bc[:], bo.broadcast_to([128, D]))
                       if first else None)

    # phase A: projections + batch-0 attention (single stream)
    with (
        tc.tile_pool(name="ppA", bufs=2, space="PSUM") as ppA,
        tc.tile_pool(name="psA", bufs=2, space="PSUM") as psA,
        tc.tile_pool(name="pzA", bufs=1, space="PSUM") as pzA,
    ):
        P["pp"] = ppA
        P["pptag"] = "p"
        P["ps"] = psA
        P["pz"] = pzA
        P["bc"] = ppA
        P["bctag"] = "p"
        proj_chunk(0)
        beats_b0 = (
            list(attn_chunk_beats(0, 0, "A")) + list(attn_chunk_beats(0, 1, "A"))
            + list(attn_chunk_beats(0, 2, "A")) + list(attn_chunk_beats(0, 3, "A"))
        )
        run_weave(beats_b0, list(proj_fillers_a()))

    # exchange batch-0 Z; overlaps with batch-1 attention below
    a2a(cc_in0, cc_out0)

    # phase B: batch-1 attention woven with part of the batch-0 output
    # projection; the rest of it is issued post-weave so it covers the
    # final collective's latency, then the batch-1 projection closes out
    with (
        tc.tile_pool(name="psB", bufs=2, space="PSUM") as psB,
        tc.tile_pool(name="pzB", bufs=1, space="PSUM") as pzB,
        tc.tile_pool(name="poB", bufs=2, space="PSUM") as poB,
    ):
        P["ps"] = psB
        P["pz"] = pzB
        P["po"] = poB
        P["bc"] = poB
        P["bctag"] = "o"

        zt2a = persist.tile([128, 8, 256], BF16, tag="zt2a", name="zt2a_sb")
        zt2b = persist.tile([128, 8, 256], BF16, tag="zt2b", name="zt2b_sb")
        ccr0 = cc_out0.rearrange("i p s -> p i s")
        ccr1 = cc_out1.rearrange("i p s -> p i s")

        def oproj_piece(zt2, st, e, rb, sink):
            """Returns (chain closure, finish closure): the finish adds the
            bias off-PE and writes its half-row-block straight out."""
            def chain():
                sink[f"ps{st}{e}"] = oproj_chain(zt2, st, e, f"o{rb}{st}{e}")
            def fin():
                if e == 0:
                    sink[f"sb{st}"] = work.tile(
                        [128, 1024], F32, tag="o", name=f"os{rb}{st}", bufs=2)
                sb = sink[f"sb{st}"]
                nc.vector.tensor_add(
                    sb[:, bass.ts(e, 512)], sink.pop(f"ps{st}{e}")[:],
                    bo_bc[:, bass.ts(e, 512)])
                nc.sync.dma_start(
                    out[rb + 128 * st:rb + 128 * st + 128, bass.ts(e, 512)],
                    sb[:, bass.ts(e, 512)])
            return chain, fin

        sink = {}
        c00, p00 = oproj_piece(zt2a, 0, 0, 0, sink)
        c01, p01 = oproj_piece(zt2a, 0, 1, 0, sink)
        c10, p10 = oproj_piece(zt2a, 1, 0, 0, sink)
        c11, p11 = oproj_piece(zt2a, 1, 1, 0, sink)

        fillers = [None] * 12 + [
            lambda: nc.sync.dma_start(zt2a[:, 0:4, :], ccr0[:, 0:4, :]),
            lambda: nc.sync.dma_start(zt2a[:, 4:8, :], ccr0[:, 4:8, :]),
            c00, p00, c01, p01,
        ]

        beats_b1 = (
            list(attn_chunk_beats(1, 0, "A")) + list(attn_chunk_beats(1, 1, "A"))
            + list(attn_chunk_beats(1, 2, "A")) + list(attn_chunk_beats(1, 3, "A"))
        )
        run_weave(beats_b1, fillers)

        # ---- tail ----
        # batch-1 exchange launches as soon as its last pieces are staged;
        # the remaining batch-0 projection chains keep the PE busy meanwhile
        a2a(cc_in1, cc_out1)
        c10(); p10(); c11(); p11()
        nc.sync.dma_start(zt2b[:, 0:4, :], ccr1[:, 0:4, :])
        nc.sync.dma_start(zt2b[:, 4:8, :], ccr1[:, 4:8, :])
        for st in (0, 1):
            cb0, pb0 = oproj_piece(zt2b, st, 0, 256, sink)
            cb1, pb1 = oproj_piece(zt2b, st, 1, 256, sink)
            cb0(); pb0(); cb1(); pb1()


def _prep_inputs(inputs, Wq, Wk, Wv, Wo, bo):
    bf16 = ml_dtypes.bfloat16
    x = np.asarray(inputs, dtype=np.float32).reshape(BS, D)
    xT = np.ascontiguousarray(x.T).astype(bf16)
    Wq = np.asarray(Wq, dtype=np.float32).astype(bf16)
    Wk = np.asarray(Wk, dtype=np.float32).astype(bf16)
    Wv = np.asarray(Wv, dtype=np.float32).astype(bf16)
    Wo = np.ascontiguousarray(np.asarray(Wo, dtype=np.float32)).astype(bf16)
    bo = np.asarray(bo, dtype=np.float32).reshape(1, D)
    ident = np.eye(128, dtype=np.float32).astype(bf16)
    triu01 = np.triu(np.ones((128, 128), dtype=np.float32)).astype(bf16)
    onesr = np.ones((128, 128), dtype=np.float32)
    in_maps = []
    for c in range(N_CORES):
        csl = slice(CD * c, CD * (c + 1))
        in_maps.append({
            "xT": xT,
            "wq": np.ascontiguousarray(Wq[:, csl]),
            "wk": np.ascontiguousarray(Wk[:, csl]),
            "wv": np.ascontiguousarray(Wv[:, csl]),
            "wo": Wo,
            "bo": bo,
            "ident": ident,
            "triu01": triu01,
            "onesr": onesr,
        })
    return in_maps


def kernel(inputs, Wq, Wk, Wv, Wo, bo):
    if "nc" not in _CACHE:
        _CACHE["nc"] = build_nc()
    nc = _CACHE["nc"]
    in_maps = _prep_inputs(inputs, Wq, Wk, Wv, Wo, bo)
    res = None
    for attempt in range(3):
        try:
            res = run_bass_kernel_spmd(nc, in_maps, core_ids=list(range(N_CORES)))
            break
        except Exception:
            if attempt == 2:
                raise
            import time as _time

            _time.sleep(5.0)
    full = np.empty((BS, D), dtype=np.float32)
    for p in range(N_CORES):
        o = res.results[p]["out"]
        t0 = 512 * (p // 2) + 256 * (p % 2)
        full[t0:t0 + 256] = o[0:256]
        full[2048 + t0:2048 + t0 + 256] = o[256:512]
    return full.reshape(B, S, D)


# revision 12
# speedup vs baseline: 1.6542x; 1.6542x over previous
"""Multi-head self-attention (B=2, S=2048, D=1024, H=16, causal) on 8 TRN2 cores.

Sharding: tensor-parallel over heads. Core c owns heads {2c, 2c+1}:
  - Wq/Wk/Wv column-sharded: core c gets columns [128c, 128c+128).
  - All matmul inputs are bf16 (f32 PSUM accumulation): rel-err ~4e-3 vs the
    f32 reference, well inside the 2e-2 gate, and it halves DMA/SBUF traffic.
  - Each core computes Q^T,K^T,V^T (head-dim on partitions) for its heads,
    both batches; V is PE-transposed back to seq-on-partitions layout.
  - Attention in transposed-scores layout: S^T[k, q] tiles, so softmax
    denominators come free from an extra ones-column in V (row 64 of the
    AV psum accumulates sum_k P^T[k, q]).  Causal masking is a 0/1 vector
    multiply on the diagonal P^T tiles (DVE) instead of a -BIG mask matmul.
  - Z^T is exchanged via TWO AllToAlls so the output projection overlaps
    with attention compute: batch-0 Z (8 pieces of [128 x 256]) right after
    batch-0 attention — its output projection weaves into batch-1 attention;
    batch-1 Z at the very end, its collective latency covered by the
    remaining batch-0 projection chains.  Core p receives tokens
    [512(p//2)+256(p%2), +256) of each batch; it projects them with full Wo
    rows (no all-reduce); out rows = [256 b0 | 256 b1].
"""

import ml_dtypes
import numpy as np

import concourse.bass as bass
import concourse.mybir as mybir
import concourse.tile as tile
from concourse import bacc
from concourse.bass_utils import run_bass_kernel_spmd

N_CORES = 8
B, S, D = 2, 2048, 1024
H = 16
HD = D // H          # 64
BS = B * S           # 4096 flattened tokens
CD = 2 * HD          # 128 head-dims per core
NM = S // 512        # 4 q-chunks per batch
SCALE = 1.0 / np.sqrt(HD)

F32 = mybir.dt.float32
BF16 = mybir.dt.bfloat16
F32R = mybir.dt.float32r
EXP = mybir.ActivationFunctionType.Exp

_CACHE = {}


def build_nc(with_collective=True, reps=1):
    nc = bacc.Bacc("TRN2", target_bir_lowering=False, debug=False, num_devices=N_CORES)

    xT = nc.dram_tensor("xT", [D, BS], BF16, kind="ExternalInput").ap()
    wq = nc.dram_tensor("wq", [D, CD], BF16, kind="ExternalInput").ap()
    wk = nc.dram_tensor("wk", [D, CD], BF16, kind="ExternalInput").ap()
    wv = nc.dram_tensor("wv", [D, CD], BF16, kind="ExternalInput").ap()
    wo = nc.dram_tensor("wo", [D, D], BF16, kind="ExternalInput").ap()
    bo = nc.dram_tensor("bo", [1, D], F32, kind="ExternalInput").ap()
    ident = nc.dram_tensor("ident", [128, 128], BF16, kind="ExternalInput").ap()
    triu01 = nc.dram_tensor("triu01", [128, 128], BF16, kind="ExternalInput").ap()
    onesr = nc.dram_tensor("onesr", [128, 128], F32, kind="ExternalInput").ap()
    out = nc.dram_tensor("out", [512, D], F32, kind="ExternalOutput").ap()

    with tile.TileContext(nc) as tc:
        with (
            tc.tile_pool(name="const", bufs=1) as constp,
            tc.tile_pool(name="persist", bufs=1) as persist,
            tc.tile_pool(name="xt", bufs=2) as xtp,
            tc.tile_pool(name="work", bufs=3) as work,
            tc.tile_pool(name="dram", bufs=1, space="DRAM") as dram,
        ):
            # ---- small constants (loaded once) ----
            ident_sb = constp.tile([128, 128], BF16)
            triu_sb = constp.tile([128, 128], BF16)
            ones_sb = constp.tile([128, 128], F32R)
            onesb_sb = constp.tile([128, 64], BF16)
            bo_bc = constp.tile([128, 1024], F32)
            cc_in0 = dram.tile([8, 128, 256], BF16)
            cc_out0 = dram.tile([8, 128, 256], BF16)
            cc_in1 = dram.tile([8, 128, 256], BF16)
            cc_out1 = dram.tile([8, 128, 256], BF16)
            xTr = xT.rearrange("(e p) s -> p e s", p=128)

            for _rep in range(reps):
                _body(nc, tc, constp, persist, xtp, work, dram,
                      xTr, wq, wk, wv, wo, bo, out,
                      ident_sb, triu_sb, ones_sb, onesb_sb, bo_bc,
                      (cc_in0, cc_out0, cc_in1, cc_out1),
                      with_collective, (ident, triu01, onesr),
                      first=(_rep == 0))

    nc.compile()
    return nc


def _body(nc, tc, constp, persist, xtp, work, dram,
          xTr, wq, wk, wv, wo, bo, out,
          ident_sb, triu_sb, ones_sb, onesb_sb, bo_bc, ccs,
          with_collective, const_srcs, first=True):
    cc_in0, cc_out0, cc_in1, cc_out1 = ccs
    r = F32R

    # ---- projection weights ----
    wq_sb = constp.tile([128, 8, CD], BF16, tag="wq", name="wq_sb")
    wk_sb = constp.tile([128, 8, CD], BF16, tag="wk", name="wk_sb")
    wv_sb = constp.tile([128, 8, CD], BF16, tag="wv", name="wv_sb")
    wqr = wq.rearrange("(e p) c -> p e c", p=128)
    # stage only the first contraction block of Wq so the first matmul can
    # start as soon as ~64KB has landed; the rest follows right behind
    nc.sync.dma_start(wq_sb[:, 0:1, :], wqr[:, 0:1, :])

    # ---- persistent activations ----
    qt_sb = persist.tile([128, BS], BF16, tag="qt", name="qt_sb")
    kt_sb = persist.tile([128, BS], BF16, tag="kt", name="kt_sb")
    v_sb = persist.tile([128, 32, 130], BF16, tag="v", name="v_sb")

    P = {}  # current-phase psum pools

    def v_transposes(sc, vt_t):
        for st in range(4):
            tt = 4 * sc + st
            v_ps = P["pp"].tile([128, 128], BF16, tag=P["pptag"], name=f"vtp{sc}{st}")
            nc.tensor.transpose(
                v_ps[:], vt_t[:, 128 * st:128 * st + 128], ident_sb[:],
            )
            nc.vector.tensor_copy(v_sb[:, tt, 0:64], v_ps[:, 0:64])
            nc.vector.tensor_copy(v_sb[:, tt, 65:129], v_ps[:, 64:128])

    def proj_parts(sc):
        """Yield fine-grained projection closures for one 512-token chunk."""
        sl = bass.ts(sc, 512)
        state = {}

        def load():
            xt_t = xtp.tile([128, 8, 512], BF16, tag="xt", name=f"xt{sc}")
            if sc == 0:
                # interleave x and weight slices so the Q chain streams
                wkr = wk.rearrange("(e p) c -> p e c", p=128)
                wvr = wv.rearrange("(e p) c -> p e c", p=128)
                nc.sync.dma_start(xt_t[:, 0:1, :], xTr[:, 0:1, sl])
                nc.sync.dma_start(wq_sb[:, 1:8, :], wqr[:, 1:8, :])
                nc.sync.dma_start(xt_t[:, 1:4, :], xTr[:, 1:4, sl])
                nc.sync.dma_start(wk_sb[:, 0:1, :], wkr[:, 0:1, :])
                nc.sync.dma_start(wv_sb[:, 0:1, :], wvr[:, 0:1, :])
                nc.sync.dma_start(xt_t[:, 4:8, :], xTr[:, 4:8, sl])
                nc.sync.dma_start(wk_sb[:, 1:8, :], wkr[:, 1:8, :])
                nc.sync.dma_start(wv_sb[:, 1:8, :], wvr[:, 1:8, :])
                if first:
                    ident_d, triu_d, onesr_d = const_srcs
                    nc.sync.dma_start(ident_sb[:], ident_d)
                    nc.sync.dma_start(triu_sb[:], triu_d)
                    nc.sync.dma_start(ones_sb[:], onesr_d.bitcast(r))
                    nc.vector.tensor_copy(onesb_sb[:], ones_sb[:, 0:64])
                    nc.vector.tensor_copy(v_sb[:, :, 64], onesb_sb[:, 0:32])
                    nc.vector.tensor_copy(v_sb[:, :, 129], onesb_sb[:, 0:32])
            else:
                nc.sync.dma_start(xt_t[:, 0:4, :], xTr[:, 0:4, sl])
                nc.sync.dma_start(xt_t[:, 4:8, :], xTr[:, 4:8, sl])
            state["xt"] = xt_t
            state["vt"] = xtp.tile([128, 512], BF16, tag="vtc", name=f"vtc{sc}")

        def group(w_sb, o_ap_fn, name):
            def run():
                p_ps = P["pp"].tile([128, 512], F32, tag=P["pptag"], name=f"pp{sc}{name}")
                for e in range(8):
                    nc.tensor.matmul(
                        p_ps[:], w_sb[:, e, :], state["xt"][:, e, :],
                        start=(e == 0), stop=(e == 7),
                    )
                with nc.allow_low_precision(reason="bf16 activations within tolerance"):
                    nc.vector.tensor_copy(o_ap_fn(), p_ps[:])
            return run

        yield load
        yield group(wq_sb, lambda: qt_sb[:, sl], "q")
        yield group(wk_sb, lambda: kt_sb[:, sl], "k")
        yield group(wv_sb, lambda: state["vt"][:], "v")
        yield lambda: v_transposes(sc, state["vt"][:])

    def proj_chunk(sc):
        for part in proj_parts(sc):
            part()

    def attn_chunk_beats(b, m, stream):
        """Yield one closure per beat; caller weaves streams together."""
        q0 = 2048 * b + 512 * m
        last_t = 4 * m + 3
        state = {}

        def beat(t):
            if t == 0:
                state["z"] = [
                    P["pz"].tile([65, 512], F32, tag=f"z{stream}{h}",
                                 name=f"z{b}{m}{h}", bufs=1)
                    for h in (0, 1)
                ]
            z_ps = state["z"]

            def av(ta, pt_sb):
                joa = max(0, 128 * (ta - 4 * m))
                for h in (0, 1):
                    nc.tensor.matmul(
                        z_ps[h][:, joa:512],
                        v_sb[:, 16 * b + ta, 65 * h:65 * h + 65],
                        pt_sb[:, 512 * h + joa:512 * h + 512],
                        start=(ta == 0), stop=(ta == last_t),
                    )

            k0 = 2048 * b + 128 * t
            jo = max(0, 128 * (t - 4 * m))
            pt_sb = work.tile([128, 1024], BF16, tag="pt", name=f"pt{b}{m}{t}", bufs=6)
            s_tiles = [
                P["ps"].tile([128, 512], F32, tag=f"s{h}", name=f"s{b}{m}{t}{h}")
                for h in (0, 1)
            ]
            for h in (0, 1):
                hsl = slice(64 * h, 64 * h + 64)
                nc.tensor.matmul(
                    s_tiles[h][:, jo:512],
                    kt_sb[hsl, k0:k0 + 128],
                    qt_sb[hsl, q0 + jo:q0 + 512],
                    start=True, stop=True,
                )
                with nc.allow_low_precision(reason="bf16 attn probs within tolerance"):
                    nc.scalar.activation(
                        pt_sb[:, 512 * h + jo:512 * h + 512], s_tiles[h][:, jo:512],
                        EXP, scale=float(SCALE),
                    )
                if t >= 4 * m:
                    # causal mask on the diagonal 128x128 tile: zero k>q
                    with nc.allow_low_precision(reason="bf16 attn probs"):
                        nc.vector.tensor_mul(
                            pt_sb[:, 512 * h + jo:512 * h + jo + 128],
                            pt_sb[:, 512 * h + jo:512 * h + jo + 128],
                            triu_sb[:],
                        )
            pend = state.setdefault("pend", [])
            pend.append((t, pt_sb))
            if len(pend) > 2:
                av(*pend.pop(0))
            if t == last_t:
                while pend:
                    av(*pend.pop(0))
                # late batch-1 norms use the PE-broadcast path: a DMA
                # round-trip there would delay the tail collective
                _norm(b, m, z_ps, fast=(b == 1 and m >= 2))

        for t in range(last_t + 1):
            yield lambda t=t: beat(t)

    def _norm(b, m, z_ps, fast=False):
        # normalize and stage for all-to-all; copy psum out (incl. denom row)
        # immediately to release the z banks, then finish from SBUF
        zcp = [work.tile([65, 512], F32, tag=f"zc{h}", name=f"zc{b}{m}{h}", bufs=2)
               for h in (0, 1)]
        for h in (0, 1):
            nc.vector.tensor_copy(zcp[h][:], z_ps[h][:])
        zt_sb = work.tile([128, 512], BF16, tag="zt", name=f"zt{b}{m}", bufs=2)
        for h in (0, 1):
            recip = work.tile([65, 512], r, tag="rc", name=f"rc{b}{m}{h}", bufs=2)
            with nc.allow_low_precision(reason="f32r is bitwise f32 here"):
                nc.vector.reciprocal(recip[64:65, :], zcp[h][64:65, :].bitcast(r))
            bc_sb = work.tile([64, 512], F32, tag="bc", name=f"bcs{b}{m}{h}", bufs=2)
            if fast:
                bc_ps = P["bc"].tile([64, 512], F32, tag=P["bctag"], name=f"bcp{b}{m}{h}")
                nc.tensor.matmul(
                    bc_ps[:], ones_sb[64:65, 0:64], recip[64:65, :],
                    start=True, stop=True,
                )
                nc.vector.tensor_copy(bc_sb[:], bc_ps[:])
            else:
                r_dram = dram.tile([1, 512], F32, tag="rd", name=f"rd{b}{m}{h}", bufs=2)
                nc.sync.dma_start(r_dram[:], recip[64:65, :].bitcast(F32))
                nc.sync.dma_start(bc_sb[:], r_dram.broadcast_to([64, 512]))
            with nc.allow_low_precision(reason="bf16 z within tolerance"):
                nc.vector.tensor_mul(
                    zt_sb[64 * h:64 * h + 64, :], zcp[h][0:64, :], bc_sb[:]
                )
        # stage pieces for the all-to-all
        cc = cc_in0 if b == 0 else cc_in1
        nc.sync.dma_start(
            cc[2 * m:2 * m + 2].rearrange("u p s -> p u s"),
            zt_sb[:].rearrange("p (u s) -> p u s", u=2),
        )

    def a2a(cin, cout):
        if with_collective:
            nc.gpsimd.collective_compute(
                "AllToAll",
                mybir.AluOpType.bypass,
                replica_groups=[list(range(N_CORES))],
                ins=[cin.opt()],
                outs=[cout.opt()],
            )
        else:
            nc.sync.dma_start(cout[:], cin[:])

    # ---- output projection pieces ----
    wo_sb = persist.tile([128, 8, D], BF16, tag="wo", name="wo_sb")

    def oproj_chain(zt2, st, e, psname):
        o_ps = P["po"].tile([128, 512], F32, tag="o", name=psname)
        for i in range(8):
            nc.tensor.matmul(
                o_ps[:],
                zt2[:, i, 128 * st:128 * st + 128],
                wo_sb[:, i, bass.ts(e, 512)],
                start=(i == 0), stop=(i == 7),
            )
        return o_ps

    def run_weave(beats, fillers):
        fi = iter(fillers)
        for bt in beats:
            bt()
            f = next(fi, None)
            if f is not None:
                f()
        for f in fi:
            if f is not None:
                f()

    def proj_fillers_a():
        for sc in range(1, 8):
            yield from proj_parts(sc)
        yield lambda: nc.sync.dma_start(
            wo_sb[:], wo.rearrange("(i p) e -> p i e", p=128))
        yield lambda: (nc.sync.dma_start(bo_bc[:], bo.broadcast_to([128, D]))
                       if first else None)

    # phase A: projections + batch-0 attention (single stream)
    with (
        tc.tile_pool(name="ppA", bufs=2, space="PSUM") as ppA,
        tc.tile_pool(name="psA", bufs=2, space="PSUM") as psA,
        tc.tile_pool(name="pzA", bufs=1, space="PSUM") as pzA,
    ):
        P["pp"] = ppA
        P["pptag"] = "p"
        P["ps"] = psA
        P["pz"] = pzA
        P["bc"] = ppA
        P["bctag"] = "p"
        proj_chunk(0)
        beats_b0 = (
            list(attn_chunk_beats(0, 0, "A")) + list(attn_chunk_beats(0, 1, "A"))
            + list(attn_chunk_beats(0, 2, "A")) + list(attn_chunk_beats(0, 3, "A"))
        )
        run_weave(beats_b0, list(proj_fillers_a()))

    # exchange batch-0 Z; overlaps with batch-1 attention below
    a2a(cc_in0, cc_out0)

    # phase B: batch-1 attention woven with part of the batch-0 output
    # projection; the rest of it is issued post-weave so it covers the
    # final collective's latency, then the batch-1 projection closes out
    with (
        tc.tile_pool(name="psB", bufs=2, space="PSUM") as psB,
        tc.tile_pool(name="pzB", bufs=1, space="PSUM") as pzB,
        tc.tile_pool(name="poB", bufs=2, space="PSUM") as poB,
    ):
        P["ps"] = psB
        P["pz"] = pzB
        P["po"] = poB
        P["bc"] = poB
        P["bctag"] = "o"

        zt2a = persist.tile([128, 8, 256], BF16, tag="zt2a", name="zt2a_sb")
        zt2b = persist.tile([128, 8, 256], BF16, tag="zt2b", name="zt2b_sb")
        ccr0 = cc_out0.rearrange("i p s -> p i s")
        ccr1 = cc_out1.rearrange("i p s -> p i s")

        def oproj_piece(zt2, st, e, rb, sink):
            """Returns (chain closure, finish closure): the finish adds the
            bias off-PE and writes its half-row-block straight out."""
            def chain():
                sink[f"ps{st}{e}"] = oproj_chain(zt2, st, e, f"o{rb}{st}{e}")
            def fin():
                if e == 0:
                    sink[f"sb{st}"] = work.tile(
                        [128, 1024], F32, tag="o", name=f"os{rb}{st}", bufs=2)
                sb = sink[f"sb{st}"]
                nc.vector.tensor_add(
                    sb[:, bass.ts(e, 512)], sink.pop(f"ps{st}{e}")[:],
                    bo_bc[:, bass.ts(e, 512)])
                nc.sync.dma_start(
                    out[rb + 128 * st:rb + 128 * st + 128, bass.ts(e, 512)],
                    sb[:, bass.ts(e, 512)])
            return chain, fin

        sink = {}
        c00, p00 = oproj_piece(zt2a, 0, 0, 0, sink)
        c01, p01 = oproj_piece(zt2a, 0, 1, 0, sink)
        c10, p10 = oproj_piece(zt2a, 1, 0, 0, sink)
        c11, p11 = oproj_piece(zt2a, 1, 1, 0, sink)

        fillers = [None] * 12 + [
            lambda: nc.sync.dma_start(zt2a[:, 0:4, :], ccr0[:, 0:4, :]),
            lambda: nc.sync.dma_start(zt2a[:, 4:8, :], ccr0[:, 4:8, :]),
            c00, p00, c01, p01,
        ]

        beats_b1 = (
            list(attn_chunk_beats(1, 0, "A")) + list(attn_chunk_beats(1, 1, "A"))
            + list(attn_chunk_beats(1, 2, "A")) + list(attn_chunk_beats(1, 3, "A"))
        )
        run_weave(beats_b1, fillers)

        # ---- tail ----
        # batch-1 exchange launches as soon as its last pieces are staged;
        # the remaining batch-0 projection chains keep the PE busy meanwhile
        a2a(cc_in1, cc_out1)
        c10(); p10(); c11(); p11()
        nc.sync.dma_start(zt2b[:, 0:4, :], ccr1[:, 0:4, :])
        nc.sync.dma_start(zt2b[:, 4:8, :], ccr1[:, 4:8, :])
        for st in (0, 1):
            cb0, pb0 = oproj_piece(zt2b, st, 0, 256, sink)
            cb1, pb1 = oproj_piece(zt2b, st, 1, 256, sink)
            cb0(); pb0(); cb1(); pb1()


def _prep_inputs(inputs, Wq, Wk, Wv, Wo, bo):
    bf16 = ml_dtypes.bfloat16
    x = np.asarray(inputs, dtype=np.float32).reshape(BS, D)
    xT = np.ascontiguousarray(x.T).astype(bf16)
    Wq = np.asarray(Wq, dtype=np.float32).astype(bf16)
    Wk = np.asarray(Wk, dtype=np.float32).astype(bf16)
    Wv = np.asarray(Wv, dtype=np.float32).astype(bf16)
    Wo = np.ascontiguousarray(np.asarray(Wo, dtype=np.float32)).astype(bf16)
    bo = np.asarray(bo, dtype=np.float32).reshape(1, D)
    ident = np.eye(128, dtype=np.float32).astype(bf16)
    triu01 = np.triu(np.ones((128, 128), dtype=np.float32)).astype(bf16)
    onesr = np.ones((128, 128), dtype=np.float32)
    in_maps = []
    for c in range(N_CORES):
        csl = slice(CD * c, CD * (c + 1))
        in_maps.append({
            "xT": xT,
            "wq": np.ascontiguousarray(Wq[:, csl]),
            "wk": np.ascontiguousarray(Wk[:, csl]),
            "wv": np.ascontiguousarray(Wv[:, csl]),
            "wo": Wo,
            "bo": bo,
            "ident": ident,
            "triu01": triu01,
            "onesr": onesr,
        })
    return in_maps


def kernel(inputs, Wq, Wk, Wv, Wo, bo):
    if "nc" not in _CACHE:
        _CACHE["nc"] = build_nc()
    nc = _CACHE["nc"]
    in_maps = _prep_inputs(inputs, Wq, Wk, Wv, Wo, bo)
    res = None
    for attempt in range(3):
        try:
            res = run_bass_kernel_spmd(nc, in_maps, core_ids=list(range(N_CORES)))
            break
        except Exception:
            if attempt == 2:
                raise
            import time as _time

            _time.sleep(5.0)
    full = np.empty((BS, D), dtype=np.float32)
    for p in range(N_CORES):
        o = res.results[p]["out"]
        t0 = 512 * (p // 2) + 256 * (p % 2)
        full[t0:t0 + 256] = o[0:256]
        full[2048 + t0:2048 + t0 + 256] = o[256:512]
    return full.reshape(B, S, D)


# revision 13
# speedup vs baseline: 1.6826x; 1.0172x over previous
"""Multi-head self-attention (B=2, S=2048, D=1024, H=16, causal) on 8 TRN2 cores.

Sharding: tensor-parallel over heads. Core c owns heads {2c, 2c+1}:
  - Wq/Wk/Wv column-sharded: core c gets columns [128c, 128c+128).
  - All matmul inputs are bf16 (f32 PSUM accumulation): rel-err ~4e-3 vs the
    f32 reference, well inside the 2e-2 gate, and it halves DMA/SBUF traffic.
  - Each core computes Q^T,K^T,V^T (head-dim on partitions) for its heads,
    both batches; V is PE-transposed back to seq-on-partitions layout.
  - Attention in transposed-scores layout: S^T[k, q] tiles, so softmax
    denominators come free from an extra ones-column in V (row 64 of the
    AV psum accumulates sum_k P^T[k, q]).  Causal masking is a 0/1 vector
    multiply on the diagonal P^T tiles (DVE) instead of a -BIG mask matmul.
  - Z^T is exchanged via TWO AllToAlls so the output projection overlaps
    with attention compute: batch-0 Z (8 pieces of [128 x 256]) right after
    batch-0 attention — its output projection weaves into batch-1 attention;
    batch-1 Z at the very end, its collective latency covered by the
    remaining batch-0 projection chains.  Core p receives tokens
    [512(p//2)+256(p%2), +256) of each batch; it projects them with full Wo
    rows (no all-reduce); out rows = [256 b0 | 256 b1].
"""

import ml_dtypes
import numpy as np

import concourse.bass as bass
import concourse.mybir as mybir
import concourse.tile as tile
from concourse import bacc
from concourse.bass_utils import run_bass_kernel_spmd

N_CORES = 8
B, S, D = 2, 2048, 1024
H = 16
HD = D // H          # 64
BS = B * S           # 4096 flattened tokens
CD = 2 * HD          # 128 head-dims per core
NM = S // 512        # 4 q-chunks per batch
SCALE = 1.0 / np.sqrt(HD)

F32 = mybir.dt.float32
BF16 = mybir.dt.bfloat16
F32R = mybir.dt.float32r
EXP = mybir.ActivationFunctionType.Exp

_CACHE = {}


def build_nc(with_collective=True, reps=1):
    nc = bacc.Bacc("TRN2", target_bir_lowering=False, debug=False, num_devices=N_CORES)

    xT = nc.dram_tensor("xT", [D, BS], BF16, kind="ExternalInput").ap()
    wq = nc.dram_tensor("wq", [D, CD], BF16, kind="ExternalInput").ap()
    wk = nc.dram_tensor("wk", [D, CD], BF16, kind="ExternalInput").ap()
    wv = nc.dram_tensor("wv", [D, CD], BF16, kind="ExternalInput").ap()
    wo = nc.dram_tensor("wo", [D, D], BF16, kind="ExternalInput").ap()
    bo = nc.dram_tensor("bo", [1, D], F32, kind="ExternalInput").ap()
    ident = nc.dram_tensor("ident", [128, 128], BF16, kind="ExternalInput").ap()
    triu01 = nc.dram_tensor("triu01", [128, 128], BF16, kind="ExternalInput").ap()
    onesr = nc.dram_tensor("onesr", [128, 128], F32, kind="ExternalInput").ap()
    out = nc.dram_tensor("out", [512, D], F32, kind="ExternalOutput").ap()

    with tile.TileContext(nc) as tc:
        with (
            tc.tile_pool(name="const", bufs=1) as constp,
            tc.tile_pool(name="persist", bufs=1) as persist,
            tc.tile_pool(name="xt", bufs=2) as xtp,
            tc.tile_pool(name="work", bufs=3) as work,
            tc.tile_pool(name="dram", bufs=1, space="DRAM") as dram,
        ):
            # ---- small constants (loaded once) ----
            ident_sb = constp.tile([128, 128], BF16)
            triu_sb = constp.tile([128, 128], BF16)
            ones_sb = constp.tile([128, 128], F32R)
            onesb_sb = constp.tile([128, 64], BF16)
            bo_bc = constp.tile([128, 1024], F32)
            cc_in0 = dram.tile([8, 128, 256], BF16)
            cc_out0 = dram.tile([8, 128, 256], BF16)
            cc_in1 = dram.tile([8, 128, 256], BF16)
            cc_out1 = dram.tile([8, 128, 256], BF16)
            xTr = xT.rearrange("(e p) s -> p e s", p=128)

            for _rep in range(reps):
                _body(nc, tc, constp, persist, xtp, work, dram,
                      xTr, wq, wk, wv, wo, bo, out,
                      ident_sb, triu_sb, ones_sb, onesb_sb, bo_bc,
                      (cc_in0, cc_out0, cc_in1, cc_out1),
                      with_collective, (ident, triu01, onesr),
                      first=(_rep == 0))

    nc.compile()
    return nc


def _body(nc, tc, constp, persist, xtp, work, dram,
          xTr, wq, wk, wv, wo, bo, out,
          ident_sb, triu_sb, ones_sb, onesb_sb, bo_bc, ccs,
          with_collective, const_srcs, first=True):
    cc_in0, cc_out0, cc_in1, cc_out1 = ccs
    r = F32R

    # ---- PE p-state warmup: ramp the clock while the first loads land ----
    with tc.tile_pool(name="pwarm", bufs=1, space="PSUM") as pwarm:
        warm = work.tile([128, 512], BF16, tag="warm", name="warm_sb")
        nc.gpsimd.memset(warm[:], 1.0)
        w_ps = pwarm.tile([128, 512], F32, tag="w", name="warm_ps")
        for i in range(4):
            nc.tensor.matmul(w_ps[:], warm[:, 0:128], warm[:],
                             start=(i == 0), stop=(i == 3))

    # ---- projection weights ----
    wq_sb = constp.tile([128, 8, CD], BF16, tag="wq", name="wq_sb")
    wk_sb = constp.tile([128, 8, CD], BF16, tag="wk", name="wk_sb")
    wv_sb = constp.tile([128, 8, CD], BF16, tag="wv", name="wv_sb")
    wqr = wq.rearrange("(e p) c -> p e c", p=128)
    # stage only the first contraction block of Wq so the first matmul can
    # start as soon as ~64KB has landed; the rest follows right behind
    nc.sync.dma_start(wq_sb[:, 0:1, :], wqr[:, 0:1, :])

    # ---- persistent activations ----
    qt_sb = persist.tile([128, BS], BF16, tag="qt", name="qt_sb")
    kt_sb = persist.tile([128, BS], BF16, tag="kt", name="kt_sb")
    v_sb = persist.tile([128, 32, 130], BF16, tag="v", name="v_sb")

    P = {}  # current-phase psum pools

    def v_transposes(sc, vt_t):
        for st in range(4):
            tt = 4 * sc + st
            v_ps = P["pp"].tile([128, 128], BF16, tag=P["pptag"], name=f"vtp{sc}{st}")
            nc.tensor.transpose(
                v_ps[:], vt_t[:, 128 * st:128 * st + 128], ident_sb[:],
            )
            nc.vector.tensor_copy(v_sb[:, tt, 0:64], v_ps[:, 0:64])
            nc.vector.tensor_copy(v_sb[:, tt, 65:129], v_ps[:, 64:128])

    def proj_parts(sc):
        """Yield fine-grained projection closures for one 512-token chunk."""
        sl = bass.ts(sc, 512)
        state = {}

        def load():
            xt_t = xtp.tile([128, 8, 512], BF16, tag="xt", name=f"xt{sc}")
            if sc == 0:
                # interleave x and weight slices so the Q chain streams
                wkr = wk.rearrange("(e p) c -> p e c", p=128)
                wvr = wv.rearrange("(e p) c -> p e c", p=128)
                nc.scalar.dma_start(xt_t[:, 0:1, :], xTr[:, 0:1, sl])
                nc.sync.dma_start(wq_sb[:, 1:8, :], wqr[:, 1:8, :])
                nc.scalar.dma_start(wk_sb[:, 0:1, :], wkr[:, 0:1, :])
                nc.scalar.dma_start(wv_sb[:, 0:1, :], wvr[:, 0:1, :])
                nc.sync.dma_start(xt_t[:, 1:4, :], xTr[:, 1:4, sl])
                nc.sync.dma_start(xt_t[:, 4:8, :], xTr[:, 4:8, sl])
                nc.sync.dma_start(wk_sb[:, 1:8, :], wkr[:, 1:8, :])
                nc.sync.dma_start(wv_sb[:, 1:8, :], wvr[:, 1:8, :])
                if first:
                    ident_d, triu_d, onesr_d = const_srcs
                    nc.scalar.dma_start(ident_sb[:], ident_d)
                    nc.scalar.dma_start(triu_sb[:], triu_d)
                    nc.scalar.dma_start(ones_sb[:], onesr_d.bitcast(r))
                    nc.vector.tensor_copy(onesb_sb[:], ones_sb[:, 0:64])
                    nc.vector.tensor_copy(v_sb[:, :, 64], onesb_sb[:, 0:32])
                    nc.vector.tensor_copy(v_sb[:, :, 129], onesb_sb[:, 0:32])
            else:
                nc.sync.dma_start(xt_t[:, 0:4, :], xTr[:, 0:4, sl])
                nc.sync.dma_start(xt_t[:, 4:8, :], xTr[:, 4:8, sl])
            state["xt"] = xt_t
            state["vt"] = xtp.tile([128, 512], BF16, tag="vtc", name=f"vtc{sc}")

        def group(w_sb, o_ap_fn, name):
            def run():
                p_ps = P["pp"].tile([128, 512], F32, tag=P["pptag"], name=f"pp{sc}{name}")
                for e in range(8):
                    nc.tensor.matmul(
                        p_ps[:], w_sb[:, e, :], state["xt"][:, e, :],
                        start=(e == 0), stop=(e == 7),
                    )
                with nc.allow_low_precision(reason="bf16 activations within tolerance"):
                    nc.vector.tensor_copy(o_ap_fn(), p_ps[:])
            return run

        yield load
        yield group(wq_sb, lambda: qt_sb[:, sl], "q")
        yield group(wk_sb, lambda: kt_sb[:, sl], "k")
        yield group(wv_sb, lambda: state["vt"][:], "v")
        yield lambda: v_transposes(sc, state["vt"][:])

    def proj_chunk(sc):
        for part in proj_parts(sc):
            part()

    def attn_chunk_beats(b, m, stream):
        """Yield one closure per beat; caller weaves streams together."""
        q0 = 2048 * b + 512 * m
        last_t = 4 * m + 3
        state = {}

        def beat(t):
            if t == 0:
                state["z"] = [
                    P["pz"].tile([65, 512], F32, tag=f"z{stream}{h}",
                                 name=f"z{b}{m}{h}", bufs=1)
                    for h in (0, 1)
                ]
            z_ps = state["z"]

            def av(ta, pt_sb):
                joa = max(0, 128 * (ta - 4 * m))
                for h in (0, 1):
                    nc.tensor.matmul(
                        z_ps[h][:, joa:512],
                        v_sb[:, 16 * b + ta, 65 * h:65 * h + 65],
                        pt_sb[:, 512 * h + joa:512 * h + 512],
                        start=(ta == 0), stop=(ta == last_t),
                    )

            k0 = 2048 * b + 128 * t
            jo = max(0, 128 * (t - 4 * m))
            pt_sb = work.tile([128, 1024], BF16, tag="pt", name=f"pt{b}{m}{t}", bufs=6)
            s_tiles = [
                P["ps"].tile([128, 512], F32, tag=f"s{h}", name=f"s{b}{m}{t}{h}")
                for h in (0, 1)
            ]
            for h in (0, 1):
                hsl = slice(64 * h, 64 * h + 64)
                nc.tensor.matmul(
                    s_tiles[h][:, jo:512],
                    kt_sb[hsl, k0:k0 + 128],
                    qt_sb[hsl, q0 + jo:q0 + 512],
                    start=True, stop=True,
                )
                with nc.allow_low_precision(reason="bf16 attn probs within tolerance"):
                    nc.scalar.activation(
                        pt_sb[:, 512 * h + jo:512 * h + 512], s_tiles[h][:, jo:512],
                        EXP, scale=float(SCALE),
                    )
                if t >= 4 * m:
                    # causal mask on the diagonal 128x128 tile: zero k>q
                    with nc.allow_low_precision(reason="bf16 attn probs"):
                        nc.vector.tensor_mul(
                            pt_sb[:, 512 * h + jo:512 * h + jo + 128],
                            pt_sb[:, 512 * h + jo:512 * h + jo + 128],
                            triu_sb[:],
                        )
            pend = state.setdefault("pend", [])
            pend.append((t, pt_sb))
            if len(pend) > 2:
                av(*pend.pop(0))
            if t == last_t:
                while pend:
                    av(*pend.pop(0))
                # late batch-1 norms use the PE-broadcast path: a DMA
                # round-trip there would delay the tail collective
                _norm(b, m, z_ps, fast=(b == 1 and m >= 2))

        for t in range(last_t + 1):
            yield lambda t=t: beat(t)

    def _norm(b, m, z_ps, fast=False):
        # normalize and stage for all-to-all; copy psum out (incl. denom row)
        # immediately to release the z banks, then finish from SBUF
        zcp = [work.tile([65, 512], F32, tag=f"zc{h}", name=f"zc{b}{m}{h}", bufs=2)
               for h in (0, 1)]
        for h in (0, 1):
            nc.vector.tensor_copy(zcp[h][:], z_ps[h][:])
        zt_sb = work.tile([128, 512], BF16, tag="zt", name=f"zt{b}{m}", bufs=2)
        for h in (0, 1):
            recip = work.tile([65, 512], r, tag="rc", name=f"rc{b}{m}{h}", bufs=2)
            with nc.allow_low_precision(reason="f32r is bitwise f32 here"):
                nc.vector.reciprocal(recip[64:65, :], zcp[h][64:65, :].bitcast(r))
            bc_sb = work.tile([64, 512], F32, tag="bc", name=f"bcs{b}{m}{h}", bufs=2)
            if fast:
                bc_ps = P["bc"].tile([64, 512], F32, tag=P["bctag"], name=f"bcp{b}{m}{h}")
                nc.tensor.matmul(
                    bc_ps[:], ones_sb[64:65, 0:64], recip[64:65, :],
                    start=True, stop=True,
                )
                nc.vector.tensor_copy(bc_sb[:], bc_ps[:])
            else:
                r_dram = dram.tile([1, 512], F32, tag="rd", name=f"rd{b}{m}{h}", bufs=2)
                nc.sync.dma_start(r_dram[:], recip[64:65, :].bitcast(F32))
                nc.sync.dma_start(bc_sb[:], r_dram.broadcast_to([64, 512]))
            with nc.allow_low_precision(reason="bf16 z within tolerance"):
                nc.vector.tensor_mul(
                    zt_sb[64 * h:64 * h + 64, :], zcp[h][0:64, :], bc_sb[:]
                )
        # stage pieces for the all-to-all
        cc = cc_in0 if b == 0 else cc_in1
        nc.sync.dma_start(
            cc[2 * m:2 * m + 2].rearrange("u p s -> p u s"),
            zt_sb[:].rearrange("p (u s) -> p u s", u=2),
        )

    def a2a(cin, cout):
        if with_collective:
            nc.gpsimd.collective_compute(
                "AllToAll",
                mybir.AluOpType.bypass,
                replica_groups=[list(range(N_CORES))],
                ins=[cin.opt()],
                outs=[cout.opt()],
            )
        else:
            nc.sync.dma_start(cout[:], cin[:])

    # ---- output projection pieces ----
    wo_sb = persist.tile([128, 8, D], BF16, tag="wo", name="wo_sb")

    def oproj_chain(zt2, st, e, psname):
        o_ps = P["po"].tile([128, 512], F32, tag="o", name=psname)
        for i in range(8):
            nc.tensor.matmul(
                o_ps[:],
                zt2[:, i, 128 * st:128 * st + 128],
                wo_sb[:, i, bass.ts(e, 512)],
                start=(i == 0), stop=(i == 7),
            )
        return o_ps

    def run_weave(beats, fillers):
        fi = iter(fillers)
        for bt in beats:
            bt()
            f = next(fi, None)
            if f is not None:
                f()
        for f in fi:
            if f is not None:
                f()

    def proj_fillers_a():
        for sc in range(1, 8):
            yield from proj_parts(sc)
        yield lambda: nc.sync.dma_start(
            wo_sb[:], wo.rearrange("(i p) e -> p i e", p=128))
        yield lambda: (nc.sync.dma_start(bo_bc[:], bo.broadcast_to([128, D]))
                       if first else None)

    # phase A: projections + batch-0 attention (single stream)
    with (
        tc.tile_pool(name="ppA", bufs=2, space="PSUM") as ppA,
        tc.tile_pool(name="psA", bufs=2, space="PSUM") as psA,
        tc.tile_pool(name="pzA", bufs=1, space="PSUM") as pzA,
    ):
        P["pp"] = ppA
        P["pptag"] = "p"
        P["ps"] = psA
        P["pz"] = pzA
        P["bc"] = ppA
        P["bctag"] = "p"
        proj_chunk(0)
        beats_b0 = (
            list(attn_chunk_beats(0, 0, "A")) + list(attn_chunk_beats(0, 1, "A"))
            + list(attn_chunk_beats(0, 2, "A")) + list(attn_chunk_beats(0, 3, "A"))
        )
        run_weave(beats_b0, list(proj_fillers_a()))

    # exchange batch-0 Z; overlaps with batch-1 attention below
    a2a(cc_in0, cc_out0)

    # phase B: batch-1 attention woven with part of the batch-0 output
    # projection; the rest of it is issued post-weave so it covers the
    # final collective's latency, then the batch-1 projection closes out
    with (
        tc.tile_pool(name="psB", bufs=2, space="PSUM") as psB,
        tc.tile_pool(name="pzB", bufs=1, space="PSUM") as pzB,
        tc.tile_pool(name="poB", bufs=2, space="PSUM") as poB,
    ):
        P["ps"] = psB
        P["pz"] = pzB
        P["po"] = poB
        P["bc"] = poB
        P["bctag"] = "o"

        zt2a = persist.tile([128, 8, 256], BF16, tag="zt2a", name="zt2a_sb")
        zt2b = persist.tile([128, 8, 256], BF16, tag="zt2b", name="zt2b_sb")
        ccr0 = cc_out0.rearrange("i p s -> p i s")
        ccr1 = cc_out1.rearrange("i p s -> p i s")

        def oproj_piece(zt2, st, e, rb, sink):
            """Returns (chain closure, finish closure): the finish adds the
            bias off-PE and writes its half-row-block straight out."""
            def chain():
                sink[f"ps{st}{e}"] = oproj_chain(zt2, st, e, f"o{rb}{st}{e}")
            def fin():
                if e == 0:
                    sink[f"sb{st}"] = work.tile(
                        [128, 1024], F32, tag="o", name=f"os{rb}{st}", bufs=2)
                sb = sink[f"sb{st}"]
                nc.vector.tensor_add(
                    sb[:, bass.ts(e, 512)], sink.pop(f"ps{st}{e}")[:],
                    bo_bc[:, bass.ts(e, 512)])
                nc.sync.dma_start(
                    out[rb + 128 * st:rb + 128 * st + 128, bass.ts(e, 512)],
                    sb[:, bass.ts(e, 512)])
            return chain, fin

        sink = {}
        c00, p00 = oproj_piece(zt2a, 0, 0, 0, sink)
        c01, p01 = oproj_piece(zt2a, 0, 1, 0, sink)
        c10, p10 = oproj_piece(zt2a, 1, 0, 0, sink)
        c11, p11 = oproj_piece(zt2a, 1, 1, 0, sink)

        fillers = [None] * 12 + [
            lambda: nc.sync.dma_start(zt2a[:, 0:4, :], ccr0[:, 0:4, :]),
            lambda: nc.sync.dma_start(zt2a[:, 4:8, :], ccr0[:, 4:8, :]),
            c00, p00, c01, p01,
        ]

        # largest chunk first: the last chunk to finish (m0, 4 beats) has
        # the shortest exp->z->norm->stage tail before the final collective
        beats_b1 = (
            list(attn_chunk_beats(1, 3, "A")) + list(attn_chunk_beats(1, 2, "A"))
            + list(attn_chunk_beats(1, 1, "A")) + list(attn_chunk_beats(1, 0, "A"))
        )
        run_weave(beats_b1, fillers)

        # ---- tail ----
        # batch-1 exchange launches as soon as its last pieces are staged;
        # the remaining batch-0 projection chains keep the PE busy meanwhile
        a2a(cc_in1, cc_out1)
        c10(); p10(); c11(); p11()
        nc.sync.dma_start(zt2b[:, 0:4, :], ccr1[:, 0:4, :])
        nc.sync.dma_start(zt2b[:, 4:8, :], ccr1[:, 4:8, :])
        for st in (0, 1):
            cb0, pb0 = oproj_piece(zt2b, st, 0, 256, sink)
            cb1, pb1 = oproj_piece(zt2b, st, 1, 256, sink)
            cb0(); pb0(); cb1(); pb1()


def _prep_inputs(inputs, Wq, Wk, Wv, Wo, bo):
    bf16 = ml_dtypes.bfloat16
    x = np.asarray(inputs, dtype=np.float32).reshape(BS, D)
    xT = np.ascontiguousarray(x.T).astype(bf16)
    Wq = np.asarray(Wq, dtype=np.float32).astype(bf16)
    Wk = np.asarray(Wk, dtype=np.float32).astype(bf16)
    Wv = np.asarray(Wv, dtype=np.float32).astype(bf16)
    Wo = np.ascontiguousarray(np.asarray(Wo, dtype=np.float32)).astype(bf16)
    bo = np.asarray(bo, dtype=np.float32).reshape(1, D)
    ident = np.eye(128, dtype=np.float32).astype(bf16)
    triu01 = np.triu(np.ones((128, 128), dtype=np.float32)).astype(bf16)
    onesr = np.ones((128, 128), dtype=np.float32)
    in_maps = []
    for c in range(N_CORES):
        csl = slice(CD * c, CD * (c + 1))
        in_maps.append({
            "xT": xT,
            "wq": np.ascontiguousarray(Wq[:, csl]),
            "wk": np.ascontiguousarray(Wk[:, csl]),
            "wv": np.ascontiguousarray(Wv[:, csl]),
            "wo": Wo,
            "bo": bo,
            "ident": ident,
            "triu01": triu01,
            "onesr": onesr,
        })
    return in_maps


def kernel(inputs, Wq, Wk, Wv, Wo, bo):
    if "nc" not in _CACHE:
        _CACHE["nc"] = build_nc()
    nc = _CACHE["nc"]
    in_maps = _prep_inputs(inputs, Wq, Wk, Wv, Wo, bo)
    res = None
    for attempt in range(3):
        try:
            res = run_bass_kernel_spmd(nc, in_maps, core_ids=list(range(N_CORES)))
            break
        except Exception:
            if attempt == 2:
                raise
            import time as _time

            _time.sleep(5.0)
    full = np.empty((BS, D), dtype=np.float32)
    for p in range(N_CORES):
        o = res.results[p]["out"]
        t0 = 512 * (p // 2) + 256 * (p % 2)
        full[t0:t0 + 256] = o[0:256]
        full[2048 + t0:2048 + t0 + 256] = o[256:512]
    return full.reshape(B, S, D)


# revision 14
# speedup vs baseline: 1.9763x; 1.1746x over previous
"""Multi-head self-attention (B=2, S=2048, D=1024, H=16, causal) on 8 TRN2 cores.

Sharding: tensor-parallel over heads. Core c owns heads {2c, 2c+1}:
  - Wq/Wk/Wv column-sharded: core c gets columns [128c, 128c+128).
  - All matmul inputs are bf16 (f32 PSUM accumulation): rel-err ~4e-3 vs the
    f32 reference, well inside the 2e-2 gate, and it halves DMA/SBUF traffic.
  - Each core computes Q^T,K^T,V^T (head-dim on partitions) for its heads,
    both batches; V is PE-transposed back to seq-on-partitions layout.
  - Attention in transposed-scores layout: S^T[k, q] tiles, so softmax
    denominators come free from an extra ones-column in V (row 64 of the
    AV psum accumulates sum_k P^T[k, q]).  Causal masking is a 0/1 vector
    multiply on the diagonal P^T tiles (DVE) instead of a -BIG mask matmul.
  - Z^T is exchanged via TWO AllToAlls so the output projection overlaps
    with attention compute: batch-0 Z (8 pieces of [128 x 256]) right after
    batch-0 attention — its output projection weaves into batch-1 attention;
    batch-1 Z at the very end, its collective latency covered by the
    remaining batch-0 projection chains.  Core p receives tokens
    [512(p//2)+256(p%2), +256) of each batch; it projects them with full Wo
    rows (no all-reduce); out rows = [256 b0 | 256 b1].
"""

import ml_dtypes
import numpy as np

import concourse.bass as bass
import concourse.mybir as mybir
import concourse.tile as tile
from concourse import bacc
from concourse.bass_utils import run_bass_kernel_spmd

N_CORES = 8
B, S, D = 2, 2048, 1024
H = 16
HD = D // H          # 64
BS = B * S           # 4096 flattened tokens
CD = 2 * HD          # 128 head-dims per core
NM = S // 512        # 4 q-chunks per batch
SCALE = 1.0 / np.sqrt(HD)

F32 = mybir.dt.float32
BF16 = mybir.dt.bfloat16
F32R = mybir.dt.float32r
EXP = mybir.ActivationFunctionType.Exp

_CACHE = {}


def build_nc(with_collective=True, reps=1):
    nc = bacc.Bacc("TRN2", target_bir_lowering=False, debug=False, num_devices=N_CORES)

    xT = nc.dram_tensor("xT", [D, BS], BF16, kind="ExternalInput").ap()
    wq = nc.dram_tensor("wq", [D, CD], BF16, kind="ExternalInput").ap()
    wk = nc.dram_tensor("wk", [D, CD], BF16, kind="ExternalInput").ap()
    wv = nc.dram_tensor("wv", [D, CD], BF16, kind="ExternalInput").ap()
    wo = nc.dram_tensor("wo", [D, D], BF16, kind="ExternalInput").ap()
    bo = nc.dram_tensor("bo", [1, D], F32, kind="ExternalInput").ap()
    ident = nc.dram_tensor("ident", [128, 128], BF16, kind="ExternalInput").ap()
    triu01 = nc.dram_tensor("triu01", [128, 128], BF16, kind="ExternalInput").ap()
    onesr = nc.dram_tensor("onesr", [128, 128], F32, kind="ExternalInput").ap()
    out = nc.dram_tensor("out", [512, D], F32, kind="ExternalOutput").ap()

    with tile.TileContext(nc) as tc:
        with (
            tc.tile_pool(name="const", bufs=1) as constp,
            tc.tile_pool(name="persist", bufs=1) as persist,
            tc.tile_pool(name="xt", bufs=2) as xtp,
            tc.tile_pool(name="work", bufs=3) as work,
            tc.tile_pool(name="dram", bufs=1, space="DRAM") as dram,
        ):
            # ---- small constants (loaded once) ----
            ident_sb = constp.tile([128, 128], BF16)
            triu_sb = constp.tile([128, 128], BF16)
            ones_sb = constp.tile([128, 128], F32R)
            onesb_sb = constp.tile([128, 64], BF16)
            bo_bc = constp.tile([128, 1024], F32)
            cc_in0 = dram.tile([8, 128, 256], BF16)
            cc_out0 = dram.tile([8, 128, 256], BF16)
            cc_in1a = dram.tile([8, 128, 128], BF16)
            cc_out1a = dram.tile([8, 128, 128], BF16)
            cc_in1b = dram.tile([8, 128, 128], BF16)
            cc_out1b = dram.tile([8, 128, 128], BF16)
            xTr = xT.rearrange("(e p) s -> p e s", p=128)

            for _rep in range(reps):
                _body(nc, tc, constp, persist, xtp, work, dram,
                      xTr, wq, wk, wv, wo, bo, out,
                      ident_sb, triu_sb, ones_sb, onesb_sb, bo_bc,
                      (cc_in0, cc_out0, cc_in1a, cc_out1a, cc_in1b, cc_out1b),
                      with_collective, (ident, triu01, onesr),
                      first=(_rep == 0))

    nc.compile()
    return nc


def _body(nc, tc, constp, persist, xtp, work, dram,
          xTr, wq, wk, wv, wo, bo, out,
          ident_sb, triu_sb, ones_sb, onesb_sb, bo_bc, ccs,
          with_collective, const_srcs, first=True):
    cc_in0, cc_out0, cc_in1a, cc_out1a, cc_in1b, cc_out1b = ccs
    r = F32R

    # ---- PE p-state warmup: ramp the clock while the first loads land ----
    with tc.tile_pool(name="pwarm", bufs=1, space="PSUM") as pwarm:
        warm = work.tile([128, 512], BF16, tag="warm", name="warm_sb")
        nc.gpsimd.memset(warm[:], 1.0)
        w_ps = pwarm.tile([128, 512], F32, tag="w", name="warm_ps")
        for i in range(4):
            nc.tensor.matmul(w_ps[:], warm[:, 0:128], warm[:],
                             start=(i == 0), stop=(i == 3))

    # ---- projection weights ----
    wq_sb = constp.tile([128, 8, CD], BF16, tag="wq", name="wq_sb")
    wk_sb = constp.tile([128, 8, CD], BF16, tag="wk", name="wk_sb")
    wv_sb = constp.tile([128, 8, CD], BF16, tag="wv", name="wv_sb")
    wqr = wq.rearrange("(e p) c -> p e c", p=128)
    # stage only the first contraction block of Wq so the first matmul can
    # start as soon as ~64KB has landed; the rest follows right behind
    nc.sync.dma_start(wq_sb[:, 0:1, :], wqr[:, 0:1, :])

    # ---- persistent activations ----
    qt_sb = persist.tile([128, BS], BF16, tag="qt", name="qt_sb")
    kt_sb = persist.tile([128, BS], BF16, tag="kt", name="kt_sb")
    v_sb = persist.tile([128, 32, 130], BF16, tag="v", name="v_sb")

    P = {}  # current-phase psum pools

    def v_transposes(sc, vt_t):
        for st in range(4):
            tt = 4 * sc + st
            v_ps = P["pp"].tile([128, 128], BF16, tag=P["pptag"], name=f"vtp{sc}{st}")
            nc.tensor.transpose(
                v_ps[:], vt_t[:, 128 * st:128 * st + 128], ident_sb[:],
            )
            nc.vector.tensor_copy(v_sb[:, tt, 0:64], v_ps[:, 0:64])
            nc.vector.tensor_copy(v_sb[:, tt, 65:129], v_ps[:, 64:128])

    def proj_parts(sc):
        """Yield fine-grained projection closures for one 512-token chunk."""
        sl = bass.ts(sc, 512)
        state = {}

        def load():
            xt_t = xtp.tile([128, 8, 512], BF16, tag="xt", name=f"xt{sc}")
            if sc == 0:
                # interleave x and weight slices so the Q chain streams
                wkr = wk.rearrange("(e p) c -> p e c", p=128)
                wvr = wv.rearrange("(e p) c -> p e c", p=128)
                nc.scalar.dma_start(xt_t[:, 0:1, :], xTr[:, 0:1, sl])
                nc.sync.dma_start(wq_sb[:, 1:8, :], wqr[:, 1:8, :])
                nc.scalar.dma_start(wk_sb[:, 0:1, :], wkr[:, 0:1, :])
                nc.scalar.dma_start(wv_sb[:, 0:1, :], wvr[:, 0:1, :])
                nc.sync.dma_start(xt_t[:, 1:4, :], xTr[:, 1:4, sl])
                nc.sync.dma_start(xt_t[:, 4:8, :], xTr[:, 4:8, sl])
                nc.sync.dma_start(wk_sb[:, 1:8, :], wkr[:, 1:8, :])
                nc.sync.dma_start(wv_sb[:, 1:8, :], wvr[:, 1:8, :])
                if first:
                    ident_d, triu_d, onesr_d = const_srcs
                    nc.scalar.dma_start(ident_sb[:], ident_d)
                    nc.scalar.dma_start(triu_sb[:], triu_d)
                    nc.scalar.dma_start(ones_sb[:], onesr_d.bitcast(r))
                    nc.vector.tensor_copy(onesb_sb[:], ones_sb[:, 0:64])
                    nc.vector.tensor_copy(v_sb[:, :, 64], onesb_sb[:, 0:32])
                    nc.vector.tensor_copy(v_sb[:, :, 129], onesb_sb[:, 0:32])
            else:
                nc.sync.dma_start(xt_t[:, 0:4, :], xTr[:, 0:4, sl])
                nc.sync.dma_start(xt_t[:, 4:8, :], xTr[:, 4:8, sl])
            state["xt"] = xt_t
            state["vt"] = xtp.tile([128, 512], BF16, tag="vtc", name=f"vtc{sc}")

        def group(w_sb, o_ap_fn, name):
            def run():
                p_ps = P["pp"].tile([128, 512], F32, tag=P["pptag"], name=f"pp{sc}{name}")
                for e in range(8):
                    nc.tensor.matmul(
                        p_ps[:], w_sb[:, e, :], state["xt"][:, e, :],
                        start=(e == 0), stop=(e == 7),
                    )
                with nc.allow_low_precision(reason="bf16 activations within tolerance"):
                    nc.vector.tensor_copy(o_ap_fn(), p_ps[:])
            return run

        yield load
        yield group(wq_sb, lambda: qt_sb[:, sl], "q")
        yield group(wk_sb, lambda: kt_sb[:, sl], "k")
        yield group(wv_sb, lambda: state["vt"][:], "v")
        yield lambda: v_transposes(sc, state["vt"][:])

    def proj_chunk(sc):
        for part in proj_parts(sc):
            part()

    def attn_chunk_beats(b, m, stream):
        """Yield one closure per beat; caller weaves streams together."""
        q0 = 2048 * b + 512 * m
        last_t = 4 * m + 3
        state = {}

        def beat(t):
            if t == 0:
                state["z"] = [
                    P["pz"].tile([65, 512], F32, tag=f"z{stream}{h}",
                                 name=f"z{b}{m}{h}", bufs=1)
                    for h in (0, 1)
                ]
            z_ps = state["z"]

            def av(ta, pt_sb):
                joa = max(0, 128 * (ta - 4 * m))
                for h in (0, 1):
                    nc.tensor.matmul(
                        z_ps[h][:, joa:512],
                        v_sb[:, 16 * b + ta, 65 * h:65 * h + 65],
                        pt_sb[:, 512 * h + joa:512 * h + 512],
                        start=(ta == 0), stop=(ta == last_t),
                    )

            k0 = 2048 * b + 128 * t
            jo = max(0, 128 * (t - 4 * m))
            pt_sb = work.tile([128, 1024], BF16, tag="pt", name=f"pt{b}{m}{t}", bufs=6)
            s_tiles = [
                P["ps"].tile([128, 512], F32, tag=f"s{h}", name=f"s{b}{m}{t}{h}")
                for h in (0, 1)
            ]
            for h in (0, 1):
                hsl = slice(64 * h, 64 * h + 64)
                nc.tensor.matmul(
                    s_tiles[h][:, jo:512],
                    kt_sb[hsl, k0:k0 + 128],
                    qt_sb[hsl, q0 + jo:q0 + 512],
                    start=True, stop=True,
                )
                with nc.allow_low_precision(reason="bf16 attn probs within tolerance"):
                    nc.scalar.activation(
                        pt_sb[:, 512 * h + jo:512 * h + 512], s_tiles[h][:, jo:512],
                        EXP, scale=float(SCALE),
                    )
                if t >= 4 * m:
                    # causal mask on the diagonal 128x128 tile: zero k>q
                    with nc.allow_low_precision(reason="bf16 attn probs"):
                        nc.vector.tensor_mul(
                            pt_sb[:, 512 * h + jo:512 * h + jo + 128],
                            pt_sb[:, 512 * h + jo:512 * h + jo + 128],
                            triu_sb[:],
                        )
            pend = state.setdefault("pend", [])
            pend.append((t, pt_sb))
            if len(pend) > 2:
                av(*pend.pop(0))
            if t == last_t:
                while pend:
                    av(*pend.pop(0))
                # late batch-1 norms use the PE-broadcast path: a DMA
                # round-trip there would delay the tail collective
                _norm(b, m, z_ps, fast=(b == 1 and m >= 2))

        for t in range(last_t + 1):
            yield lambda t=t: beat(t)

    def _norm(b, m, z_ps, fast=False):
        # normalize and stage for all-to-all; copy psum out (incl. denom row)
        # immediately to release the z banks, then finish from SBUF
        zcp = [work.tile([65, 512], F32, tag=f"zc{h}", name=f"zc{b}{m}{h}", bufs=2)
               for h in (0, 1)]
        for h in (0, 1):
            nc.vector.tensor_copy(zcp[h][:], z_ps[h][:])
        zt_sb = work.tile([128, 512], BF16, tag="zt", name=f"zt{b}{m}", bufs=2)
        for h in (0, 1):
            recip = work.tile([65, 512], r, tag="rc", name=f"rc{b}{m}{h}", bufs=2)
            with nc.allow_low_precision(reason="f32r is bitwise f32 here"):
                nc.vector.reciprocal(recip[64:65, :], zcp[h][64:65, :].bitcast(r))
            bc_sb = work.tile([64, 512], F32, tag="bc", name=f"bcs{b}{m}{h}", bufs=2)
            if fast:
                bc_ps = P["bc"].tile([64, 512], F32, tag=P["bctag"], name=f"bcp{b}{m}{h}")
                nc.tensor.matmul(
                    bc_ps[:], ones_sb[64:65, 0:64], recip[64:65, :],
                    start=True, stop=True,
                )
                nc.vector.tensor_copy(bc_sb[:], bc_ps[:])
            else:
                r_dram = dram.tile([1, 512], F32, tag="rd", name=f"rd{b}{m}{h}", bufs=2)
                nc.sync.dma_start(r_dram[:], recip[64:65, :].bitcast(F32))
                nc.sync.dma_start(bc_sb[:], r_dram.broadcast_to([64, 512]))
            with nc.allow_low_precision(reason="bf16 z within tolerance"):
                nc.vector.tensor_mul(
                    zt_sb[64 * h:64 * h + 64, :], zcp[h][0:64, :], bc_sb[:]
                )
        # stage pieces for the all-to-all
        if b == 0:
            nc.sync.dma_start(
                cc_in0[2 * m:2 * m + 2].rearrange("u p s -> p u s"),
                zt_sb[:].rearrange("p (u s) -> p u s", u=2),
            )
        else:
            # chunks m2/m3 (tokens 1024:2048) go in the early exchange,
            # m0/m1 (0:1024) in the final one
            cc = cc_in1a if m >= 2 else cc_in1b
            mm = m - 2 if m >= 2 else m
            nc.sync.dma_start(
                cc[4 * mm:4 * mm + 4].rearrange("u p s -> p u s"),
                zt_sb[:].rearrange("p (u s) -> p u s", u=4),
            )

    def a2a(cin, cout):
        if with_collective:
            nc.gpsimd.collective_compute(
                "AllToAll",
                mybir.AluOpType.bypass,
                replica_groups=[list(range(N_CORES))],
                ins=[cin.opt()],
                outs=[cout.opt()],
            )
        else:
            nc.sync.dma_start(cout[:], cin[:])

    # ---- output projection pieces ----
    wo_sb = persist.tile([128, 8, D], BF16, tag="wo", name="wo_sb")

    def oproj_chain(zt2, st, e, psname):
        o_ps = P["po"].tile([128, 512], F32, tag="o", name=psname)
        for i in range(8):
            nc.tensor.matmul(
                o_ps[:],
                zt2[:, i, 128 * st:128 * st + 128],
                wo_sb[:, i, bass.ts(e, 512)],
                start=(i == 0), stop=(i == 7),
            )
        return o_ps

    def run_weave(beats, fillers):
        fi = iter(fillers)
        for bt in beats:
            bt()
            f = next(fi, None)
            if f is not None:
                f()
        for f in fi:
            if f is not None:
                f()

    def proj_fillers_a():
        for sc in range(1, 8):
            yield from proj_parts(sc)
        yield lambda: nc.sync.dma_start(
            wo_sb[:], wo.rearrange("(i p) e -> p i e", p=128))
        yield lambda: (nc.sync.dma_start(bo_bc[:], bo.broadcast_to([128, D]))
                       if first else None)

    # phase A: projections + batch-0 attention (single stream)
    with (
        tc.tile_pool(name="ppA", bufs=2, space="PSUM") as ppA,
        tc.tile_pool(name="psA", bufs=2, space="PSUM") as psA,
        tc.tile_pool(name="pzA", bufs=1, space="PSUM") as pzA,
    ):
        P["pp"] = ppA
        P["pptag"] = "p"
        P["ps"] = psA
        P["pz"] = pzA
        P["bc"] = ppA
        P["bctag"] = "p"
        proj_chunk(0)
        beats_b0 = (
            list(attn_chunk_beats(0, 0, "A")) + list(attn_chunk_beats(0, 1, "A"))
            + list(attn_chunk_beats(0, 2, "A")) + list(attn_chunk_beats(0, 3, "A"))
        )
        run_weave(beats_b0, list(proj_fillers_a()))

    # exchange batch-0 Z; overlaps with batch-1 attention below
    a2a(cc_in0, cc_out0)

    # phase B: batch-1 attention woven with part of the batch-0 output
    # projection; the rest of it is issued post-weave so it covers the
    # final collective's latency, then the batch-1 projection closes out
    with (
        tc.tile_pool(name="psB", bufs=2, space="PSUM") as psB,
        tc.tile_pool(name="pzB", bufs=1, space="PSUM") as pzB,
        tc.tile_pool(name="poB", bufs=2, space="PSUM") as poB,
    ):
        P["ps"] = psB
        P["pz"] = pzB
        P["po"] = poB
        P["bc"] = poB
        P["bctag"] = "o"

        zt2a = persist.tile([128, 8, 256], BF16, tag="zt2a", name="zt2a_sb")
        zt2c = persist.tile([128, 8, 128], BF16, tag="zt2c", name="zt2c_sb")
        zt2d = persist.tile([128, 8, 128], BF16, tag="zt2d", name="zt2d_sb")
        ccr0 = cc_out0.rearrange("i p s -> p i s")
        ccr1a = cc_out1a.rearrange("i p s -> p i s")
        ccr1b = cc_out1b.rearrange("i p s -> p i s")

        def oproj_piece(zt2, st, e, rb, sink):
            """Returns (chain closure, finish closure): the finish adds the
            bias off-PE and writes its half-row-block straight out."""
            def chain():
                sink[f"ps{st}{e}"] = oproj_chain(zt2, st, e, f"o{rb}{st}{e}")
            def fin():
                if e == 0:
                    sink[f"sb{st}"] = work.tile(
                        [128, 1024], F32, tag="o", name=f"os{rb}{st}", bufs=2)
                sb = sink[f"sb{st}"]
                nc.vector.tensor_add(
                    sb[:, bass.ts(e, 512)], sink.pop(f"ps{st}{e}")[:],
                    bo_bc[:, bass.ts(e, 512)])
                nc.sync.dma_start(
                    out[rb + 128 * st:rb + 128 * st + 128, bass.ts(e, 512)],
                    sb[:, bass.ts(e, 512)])
            return chain, fin

        sink = {}
        c00, p00 = oproj_piece(zt2a, 0, 0, 0, sink)
        c01, p01 = oproj_piece(zt2a, 0, 1, 0, sink)
        c10, p10 = oproj_piece(zt2a, 1, 0, 0, sink)
        c11, p11 = oproj_piece(zt2a, 1, 1, 0, sink)
        ca0, fa0 = oproj_piece(zt2c, 0, 0, 384, sink)
        ca1, fa1 = oproj_piece(zt2c, 0, 1, 384, sink)

        fillers = [None] * 12 + [
            lambda: nc.sync.dma_start(zt2a[:, 0:4, :], ccr0[:, 0:4, :]),
            lambda: nc.sync.dma_start(zt2a[:, 4:8, :], ccr0[:, 4:8, :]),
            c00, p00, c01, p01,
        ] + [None] * 10 + [
            # beat 27 staged m2 -> tokens [1024:2048) can exchange now
            lambda: a2a(cc_in1a, cc_out1a),
            lambda: nc.sync.dma_start(zt2c[:], ccr1a[:]),
            ca0, fa0, ca1, fa1,
        ]

        # largest chunk first: the last chunk to finish (m0, 4 beats) has
        # the shortest exp->z->norm->stage tail before the final collective
        beats_b1 = (
            list(attn_chunk_beats(1, 3, "A")) + list(attn_chunk_beats(1, 2, "A"))
            + list(attn_chunk_beats(1, 1, "A")) + list(attn_chunk_beats(1, 0, "A"))
        )
        run_weave(beats_b1, fillers)

        # ---- tail ----
        # batch-1 exchange launches as soon as its last pieces are staged;
        # the remaining batch-0 projection chains keep the PE busy meanwhile
        a2a(cc_in1b, cc_out1b)
        c10(); p10(); c11(); p11()
        nc.sync.dma_start(zt2d[:], ccr1b[:])
        cb0, pb0 = oproj_piece(zt2d, 0, 0, 256, sink)
        cb1, pb1 = oproj_piece(zt2d, 0, 1, 256, sink)
        cb0(); pb0(); cb1(); pb1()


def _prep_inputs(inputs, Wq, Wk, Wv, Wo, bo):
    bf16 = ml_dtypes.bfloat16
    x = np.asarray(inputs, dtype=np.float32).reshape(BS, D)
    xT = np.ascontiguousarray(x.T).astype(bf16)
    Wq = np.asarray(Wq, dtype=np.float32).astype(bf16)
    Wk = np.asarray(Wk, dtype=np.float32).astype(bf16)
    Wv = np.asarray(Wv, dtype=np.float32).astype(bf16)
    Wo = np.ascontiguousarray(np.asarray(Wo, dtype=np.float32)).astype(bf16)
    bo = np.asarray(bo, dtype=np.float32).reshape(1, D)
    ident = np.eye(128, dtype=np.float32).astype(bf16)
    triu01 = np.triu(np.ones((128, 128), dtype=np.float32)).astype(bf16)
    onesr = np.ones((128, 128), dtype=np.float32)
    in_maps = []
    for c in range(N_CORES):
        csl = slice(CD * c, CD * (c + 1))
        in_maps.append({
            "xT": xT,
            "wq": np.ascontiguousarray(Wq[:, csl]),
            "wk": np.ascontiguousarray(Wk[:, csl]),
            "wv": np.ascontiguousarray(Wv[:, csl]),
            "wo": Wo,
            "bo": bo,
            "ident": ident,
            "triu01": triu01,
            "onesr": onesr,
        })
    return in_maps


def kernel(inputs, Wq, Wk, Wv, Wo, bo):
    if "nc" not in _CACHE:
        _CACHE["nc"] = build_nc()
    nc = _CACHE["nc"]
    in_maps = _prep_inputs(inputs, Wq, Wk, Wv, Wo, bo)
    res = None
    for attempt in range(3):
        try:
            res = run_bass_kernel_spmd(nc, in_maps, core_ids=list(range(N_CORES)))
            break
        except Exception:
            if attempt == 2:
                raise
            import time as _time

            _time.sleep(5.0)
    full = np.empty((BS, D), dtype=np.float32)
    for p in range(N_CORES):
        o = res.results[p]["out"]
        t0 = 512 * (p // 2) + 256 * (p % 2)
        full[t0:t0 + 256] = o[0:256]
        full[2048 + 128 * p:2048 + 128 * p + 128] = o[256:384]
        full[3072 + 128 * p:3072 + 128 * p + 128] = o[384:512]
    return full.reshape(B, S, D)
